# revision 1
# baseline (speedup 1.0000x reference)
"""A16W4 grouped asymmetric dequant GEMM, column-parallel over 8 NeuronCores.

Shapes (hardcoded per problem spec):
  x:      (256, 4096)  f32
  W_q:    (14336, 4096) int32, 4-bit codes in [0,16)
  scales: (14336, 64)  f32   (group size 64 along K)
  zeros:  (14336, 64)  f32
  bias:   (14336,)     f32
  out:    (256, 14336) f32 = x @ ((W_q - zeros)*scales).T + bias

Sharding: W_q/scales/zeros/bias split along out_features into 8 shards of
1792; x replicated; per-core dequant+GEMM; concat outputs on host.
"""

import numpy as np

M, K, O, G = 256, 4096, 14336, 64
NG = K // G  # 64 groups
NC = 8
OS = O // NC  # 1792 per core


def _kernel_jax(x, W_q, scales, zeros, bias):
    import jax
    import jax.numpy as jnp

    devs = jax.devices()
    if len(devs) < NC:
        raise RuntimeError(f"need {NC} devices, have {len(devs)}")

    # Stack per-core shards: leading axis = device.
    Wq_s = W_q.reshape(NC, OS, K)
    sc_s = scales.reshape(NC, OS, NG)
    zp_s = zeros.reshape(NC, OS, NG)
    b_s = bias.reshape(NC, OS)
    x_s = np.broadcast_to(x, (NC, M, K))

    def shard_fn(xl, wq, sc, zp, bl):
        w = wq.astype(jnp.float32).reshape(OS, NG, G)
        w = (w - zp[:, :, None]) * sc[:, :, None]
        w = w.reshape(OS, K)
        return xl @ w.T + bl[None, :]

    out_s = jax.pmap(shard_fn, devices=devs[:NC])(
        x_s, Wq_s.astype(np.float32), sc_s, zp_s, b_s
    )
    return np.asarray(out_s).transpose(1, 0, 2).reshape(M, O).astype(np.float32)


def _kernel_numpy(x, W_q, scales, zeros, bias):
    out = np.empty((M, O), dtype=np.float32)
    for c in range(NC):
        lo, hi = c * OS, (c + 1) * OS
        w = W_q[lo:hi].astype(np.float32).reshape(OS, NG, G)
        w = (w - zeros[lo:hi, :, None]) * scales[lo:hi, :, None]
        out[:, lo:hi] = x @ w.reshape(OS, K).T + bias[lo:hi][None, :]
    return out


def kernel(x, W_q, scales, zeros, bias):
    x = np.asarray(x, dtype=np.float32)
    W_q = np.asarray(W_q)
    scales = np.asarray(scales, dtype=np.float32)
    zeros = np.asarray(zeros, dtype=np.float32)
    bias = np.asarray(bias, dtype=np.float32)
    try:
        return _kernel_jax(x, W_q, scales, zeros, bias)
    except Exception:
        return _kernel_numpy(x, W_q, scales, zeros, bias)

